# revision 56
# baseline (speedup 1.0000x reference)
"""Trainium2 Bass kernel for hetero-GNN (2x ResGatedGraphConv + segment-mean pooling + MLP).

Sharding: destination-node range per core; each core processes the edges whose
dst falls in its range. Host does index marshalling only; all model arithmetic
runs on device.

Device strategy ("degree rounds"):
  - dst nodes are grouped into 128-slot buckets; 8 buckets form a PSUM group
    whose aggregate [128 slots, 8*64] lives in one PSUM bank.
  - edges of a bucket are packed into R identity rounds (the j-th edge of
    slot p sits at row p of round j) plus <=F flex rounds (leftovers, with a
    host-built fp8 one-hot scatter matrix).
  - a slab = one round of each of the 8 buckets = 8 subtiles of 128 edges.
    Per slab: 8 fused matmuls [xt.T @ W_aug] (row-tiled concurrent pairs),
    one sigmoid (ACT), one gated multiply (DVE), and ONE identity-stationary
    matmul that scatter-adds all 8 subtiles into the group aggregate.
  - skip connection is folded into the same PSUM accumulation (a dummy zero
    matmul opens the accumulation group for the whole bank).
  - relu + pooling via per-bucket one-hot matmul into a [G, 2H] transposed
    pooled PSUM; AllReduce across 8 cores; small MLP head on device.
"""
import os
import sys
import types
import numpy as np
import ml_dtypes

F8NP = ml_dtypes.float8_e4m3fn

NCORES = 8
G = 128
H = 64
F = 16
NC_N = 100000
NB_N = 200000
GRP = 8          # buckets per PSUM group
LAST_EXEC_NS = None


def _install_ntff_shim():
    if 'antenv.axon_hooks' in sys.modules:
        return
    try:
        mod = types.ModuleType('antenv.axon_hooks')
        _h = [None]
        mod.set_axon_ntff_profile_hook = lambda h: _h.__setitem__(0, h)
        mod.get_axon_ntff_profile_hook = lambda: _h[0]
        sys.modules['antenv.axon_hooks'] = mod
        import antenv
        antenv.axon_hooks = mod
        from trn_agent_boot.trn_boot import _ntff_profile_via_ctypes
        mod.set_axon_ntff_profile_hook(
            _ntff_profile_via_ctypes('/opt/axon/libaxon_pjrt.so'))
    except Exception:
        pass


def _waug(ii, rel):
    Wq, Wv, Wk = ii[f"Wq_{rel}"], ii[f"Wv_{rel}"], ii[f"Wk_{rel}"]
    We = ii[f"We_{rel}"][0]
    bq, bv, bk, be = (ii[f"bq_{rel}"], ii[f"bv_{rel}"],
                      ii[f"bk_{rel}"], ii[f"be_{rel}"])
    w = np.zeros((35, 128), np.float32)
    w[0:16, 0:64] = Wq
    w[0:16, 64:128] = Wv
    w[16, 0:64] = 2 * We
    w[16, 64:128] = We
    w[17, 0:64] = bq + bk + 2 * be
    w[17, 64:128] = bv + be
    w[18:34, 0:64] = Wk
    return w


def pack_relation(xs, xd, src, dst, ea, D, ii, rel, batch_dst):
    """Host marshalling for one relation.

    Returns common schedule + per-core device arrays."""
    E = len(src)
    nbuck = (D + 127) // 128
    # per-core degree-sorted slot permutation: rank dst nodes by degree so
    # each 128-slot bucket holds near-equal degrees (minimal round padding).
    core_all = dst // D
    loc_all = dst % D
    deg_node = np.bincount(core_all * D + loc_all,
                           minlength=NCORES * D).reshape(NCORES, D)
    order_nodes = np.argsort(-deg_node, axis=1, kind="stable")  # rank->loc
    rank_of_loc = np.empty((NCORES, D), np.int64)
    np.put_along_axis(rank_of_loc, order_nodes,
                      np.broadcast_to(np.arange(D), (NCORES, D)), axis=1)
    slot_all = rank_of_loc[core_all, loc_all]
    key = core_all * D + slot_all
    order = np.argsort(key, kind="stable")
    src_s, dst_s, ea_s = src[order], dst[order], ea[order]
    key_s = key[order]
    core = core_all[order]
    buck = (key_s % D) // 128
    slot = (key_s % D) % 128
    lin = (core * nbuck + buck) * 128 + slot
    deg = np.bincount(lin, minlength=NCORES * nbuck * 128) \
            .reshape(NCORES, nbuck, 128)
    starts = np.searchsorted(key_s, key_s, side="left")
    rank = np.arange(E) - starts

    # common per-bucket-position R (identity rounds): minimize
    # R + w*max_core(F).  Flex subtiles cost more than identity rounds
    # (extra one-hot DMA + per-subtile scatter matmul), so weight them and
    # prefer the larger R on ties.
    maxd = int(deg.max())
    bestT = np.full(nbuck, np.inf)
    bestR = np.zeros(nbuck, np.int64)
    for R in range(0, maxd + 1):
        lo = np.maximum(deg - R, 0).sum(-1)            # [NCORES, nbuck]
        Fk = (-(-lo // 128)).max(0)                    # [nbuck]
        T = R + 1.8 * Fk
        upd = T <= bestT
        bestT[upd] = T[upd]
        bestR[upd] = R

    # group buckets (sorted by R desc) into chunks of GRP
    border = np.argsort(-bestR, kind="stable")
    ngroups = (nbuck + GRP - 1) // GRP
    groups = []
    bucket_group = np.zeros(nbuck, np.int64)   # bucket -> group
    bucket_pos = np.zeros(nbuck, np.int64)     # bucket -> index in group
    bucket_Rs = np.zeros(nbuck, np.int64)      # bucket -> group R*
    xt_col = 0
    oh_blk = 0
    for g in range(ngroups):
        bks = border[g * GRP:(g + 1) * GRP]
        n_b = len(bks)
        Rs = int(bestR[bks].max()) if n_b else 0
        # leftovers recomputed at group R*; order buckets by flex count so
        # each flex slab's present subtiles form a prefix
        lo2 = np.maximum(deg[:, bks, :] - Rs, 0).sum(-1)   # [NCORES, n_b]
        Fk = (-(-lo2 // 128)).max(0)                       # [n_b]
        perm = np.argsort(-Fk, kind="stable")
        bks = bks[perm]
        Fk = Fk[perm]
        Fs = int(Fk.max()) if n_b else 0
        flex = []
        oh_idx = {}
        for f in range(Fs):
            present = [(i, 0) for i in range(n_b) if Fk[i] > f]
            present = [(i, oh_blk + j) for j, (i, _) in enumerate(present)]
            for i, ob in present:
                oh_idx[(i, f)] = ob
            oh_blk += len(present)
            flex.append(present)
        bucket_group[bks] = g
        bucket_pos[bks] = np.arange(n_b)
        bucket_Rs[bks] = Rs
        groups.append({
            "n_b": n_b, "R": Rs, "F": Fs, "bks": bks, "Fk": Fk,
            "flex": flex, "oh_idx": oh_idx, "xt_off": xt_col,
        })
        xt_col += (Rs + Fs) * 512
    XC = max(xt_col, 512)
    OC = max(oh_blk * 128, 128)

    # per-edge destination column in xt (per core arrays share the schedule)
    # xt layout: [64, nsub*128]; subtile (group g, slab s, pos i) at column
    # block (xt_off/512)*8 + s*8 + i  (xt_off counts 512-col slab units).
    g_of = bucket_group[buck]
    i_of = bucket_pos[buck]
    Rs_of = bucket_Rs[buck]
    xoff_of = np.array([gr["xt_off"] for gr in groups], np.int64)[g_of]
    suboff_of = xoff_of // 512 * 8
    is_id = rank < Rs_of
    col = np.zeros(E, np.int64)
    col[is_id] = ((suboff_of[is_id] + rank[is_id] * 8 + i_of[is_id]) * 128
                  + slot[is_id])
    # flex: position among the bucket's leftover edges (dst-sorted order)
    lx = ~is_id
    lin_lx = lin[lx] // 128      # (core,bucket) linear id of leftover edges
    first = np.searchsorted(lin_lx, lin_lx, side="left")
    fpos = np.arange(lx.sum()) - first
    f_of = fpos // 128
    row = fpos % 128
    col[lx] = ((suboff_of[lx] + (Rs_of[lx] + f_of) * 8 + i_of[lx]) * 128
               + row)
    # oh block index for flex edges
    ohmap = np.full((nbuck, 32), -1, np.int64)
    for gr in groups:
        for (i, f), ob in gr["oh_idx"].items():
            ohmap[gr["bks"][i], f] = ob
    oh_of = np.zeros(E, np.int64)
    oh_of[lx] = ohmap[buck[lx], f_of]
    assert (oh_of[lx] >= 0).all()
    flexrow = np.zeros(E, np.int64)
    flexrow[lx] = row

    xsT = xs.astype(np.float32)
    xdT = xd.astype(np.float32)
    per_core = []
    cb = np.searchsorted(core, np.arange(NCORES + 1))
    for m in range(NCORES):
        s0, s1 = cb[m], cb[m + 1]
        c_src, c_dst = src_s[s0:s1], dst_s[s0:s1]
        c_ea, c_col = ea_s[s0:s1], col[s0:s1]
        c_lx = lx[s0:s1]
        c_oh = oh_of[s0:s1]
        c_fr = flexrow[s0:s1]
        c_slot = slot[s0:s1]
        xt = np.zeros((128, XC * 2), np.float32)
        xt[0:16, c_col] = xsT[c_src].T
        xt[16, c_col] = c_ea
        xt[17, c_col] = 1.0
        xt[18:34, c_col] = xdT[c_dst].T
        # flex one-hots: edge at (oh block, row) -> slot
        oh = np.zeros((128, OC), np.float32)
        oh[c_fr[c_lx], c_oh[c_lx] * 128 + c_slot[c_lx]] = 1.0
        # pa (skip lhsT, [32, nbuck*128]) + ohg (pooling one-hot),
        # columns follow the per-core degree-rank permutation
        PC = ngroups * GRP * 128
        GC = ngroups * GRP * 128
        pa = np.zeros((32, PC), np.float32)
        ohg = np.zeros((128, GC), np.float32)
        rk2loc = order_nodes[m]
        for g, gr in enumerate(groups):
            for i, k in enumerate(gr["bks"]):
                r0 = k * 128
                w = min(128, D - r0)
                if w <= 0:
                    continue
                nodes = m * D + rk2loc[r0:r0 + w]
                cblk = (g * GRP + i) * 128
                pa[0:16, cblk:cblk + w] = xdT[nodes].T
                pa[16, cblk:cblk + w] = 1.0
                bt = batch_dst[nodes]
                ohg[np.arange(w), cblk + bt] = 1.0
        f8 = np.float16 if os.environ.get("KF16") else F8NP
        per_core.append({
            "xt": xt.astype(f8),
            "oh": oh.astype(f8),
            "pa": pa.astype(np.float16),
            "ohg": ohg.astype(f8),
        })

    w2 = np.zeros((128, 128), np.float32)
    w2[0:35] = _waug(ii, rel)
    ws4 = np.zeros((32, 64), np.float32)
    ws4[0:16] = ii[f"Wskip_{rel}"]
    ws4[16] = ii[f"bconv_{rel}"]
    nslabs = sum(gr["R"] + gr["F"] for gr in groups)
    return {
        "groups": groups, "XC": XC, "OC": OC, "ngroups": ngroups,
        "per_core": per_core, "w2": w2.astype(np.float16),
        "ws4": ws4.astype(np.float16), "nslabs": nslabs, "D": D,
        "nbuck": nbuck,
    }


def pack_all(ii):
    Dc, Db = NC_N // NCORES, NB_N // NCORES
    rel_c = pack_relation(ii["x_x"], ii["x_c"], ii["src_ac"].astype(np.int64),
                          ii["dst_ac"].astype(np.int64),
                          np.asarray(ii["ea_ac"])[:, 0], Dc, ii, "ac",
                          ii["batch_c"].astype(np.int64))
    rel_b = pack_relation(ii["x_c"], ii["x_b"], ii["src_cb"].astype(np.int64),
                          ii["dst_cb"].astype(np.int64),
                          np.asarray(ii["ea_cb"])[:, 0], Db, ii, "cb",
                          ii["batch_b"].astype(np.int64))

    cnt_c = np.bincount(ii["batch_c"].astype(np.int64), minlength=G)
    cnt_b = np.bincount(ii["batch_b"].astype(np.int64), minlength=G)
    recip2 = np.zeros((G, 128), np.float32)
    recip2[:, 0:64] = (1.0 / np.maximum(cnt_c, 1))[:, None]
    recip2[:, 64:128] = (1.0 / np.maximum(cnt_b, 1))[:, None]

    mlp = {
        "W1": ii["W1"].astype(np.float16), "W2": ii["W2"].astype(np.float16),
        "W3": ii["W3"].astype(np.float16),
        "Wout": ii["Wout"].astype(np.float16),
        "b1": np.asarray(ii["b1"], np.float32).reshape(64, 1),
        "b2": np.asarray(ii["b2"], np.float32).reshape(64, 1),
        "b3": np.asarray(ii["b3"], np.float32).reshape(64, 1),
        "bout": np.asarray(ii["bout"], np.float32).reshape(1, 1),
    }
    f8 = np.float16 if os.environ.get("KF16") else F8NP
    ident8 = np.eye(128, dtype=f8)
    ident16 = np.eye(128, dtype=np.float16)
    zl = np.zeros((1, 128), np.float16)
    zr = np.zeros((1, 512), np.float16)
    z128 = np.zeros((128, 128), np.float32)
    return {"c": rel_c, "b": rel_b, "recip2": recip2.astype(np.float16),
            "mlp": mlp, "ident8": ident8, "ident16": ident16,
            "zl": zl, "zr": zr, "z128": z128}


def emulate(ii):
    """Numpy emulation of the device program (for packing validation)."""
    pk = pack_all(ii)
    pooled = np.zeros((G, 128), np.float64)
    for tag in ("c", "b"):
        rl = pk[tag]
        w2 = rl["w2"].astype(np.float32)
        ws4 = rl["ws4"].astype(np.float32)
        for m in range(NCORES):
            pc = rl["per_core"][m]
            xt = pc["xt"].astype(np.float32)
            oh = pc["oh"].astype(np.float32)
            pa = pc["pa"].astype(np.float32)
            ohg = pc["ohg"].astype(np.float32)
            for g, gr in enumerate(rl["groups"]):
                n_b, Rs, Fs = gr["n_b"], gr["R"], gr["F"]
                agg = np.zeros((128, n_b, 64), np.float32)
                for i in range(n_b):
                    cblk = (g * GRP + i) * 128
                    lhs = pa[0:17, cblk:cblk + 128]
                    agg[:, i, :] += lhs.T @ ws4[0:17]
                for s in range(Rs + Fs):
                    c0 = gr["xt_off"] * 2 + s * 1024
                    blkx = xt[:, c0:c0 + 1024]
                    sv = np.zeros((128, 8, 128), np.float32)
                    for i in range(8):
                        sv[:, i, :] = (blkx[:, i * 128:(i + 1) * 128].T @ w2)
                    gt = (1.0 / (1.0 + np.exp(-sv[:, :, 0:64]))) \
                        .astype(np.float16).astype(np.float32)
                    msg = (gt * sv[:, :, 64:128]).astype(np.float16) \
                        .astype(np.float32)
                    if s < Rs:
                        agg += msg[:, :n_b, :]
                    else:
                        for (i, ob) in gr["flex"][s - Rs]:
                            ohb = oh[:, ob * 128:(ob + 1) * 128]
                            agg[:, i, :] += ohb.T @ msg[:, i, :]
                h = np.maximum(agg, 0.0).astype(np.float16).astype(np.float32)
                off = 0 if tag == "c" else 64
                for i in range(n_b):
                    ohgb = ohg[:, (g * GRP + i) * 128:(g * GRP + i + 1) * 128]
                    pooled[:, off:off + 64] += ohgb.T @ h[:, i, :]
    mean = pooled * pk["recip2"].astype(np.float64)
    hcur = mean.T.astype(np.float32)          # [2H, G]
    mlp = pk["mlp"]
    for wk, bk in (("W1", "b1"), ("W2", "b2"), ("W3", "b3")):
        hcur = np.maximum(mlp[wk].astype(np.float32).T @ hcur + mlp[bk], 0.0)
    out = mlp["Wout"].astype(np.float32).T @ hcur + mlp["bout"]
    return out.reshape(G)


def kernel(**inputs):
    _install_ntff_shim()
    import concourse.bass as bass  # noqa: F401
    import concourse.bacc as bacc
    import concourse.mybir as mybir
    import concourse.tile as tile
    from concourse.bass_utils import run_bass_kernel_spmd

    F32 = mybir.dt.float32
    F16 = mybir.dt.float16
    FP8 = F16 if os.environ.get("KF16") else mybir.dt.float8e4
    AF = mybir.ActivationFunctionType
    OP = mybir.AluOpType

    ii = {k: np.asarray(v) for k, v in inputs.items()}
    pk = pack_all(ii)

    nc = bacc.Bacc("TRN2", target_bir_lowering=False, debug=False,
                   num_devices=NCORES)

    def din(name, arr0):
        return nc.dram_tensor(name, list(arr0.shape),
                              mybir.dt.from_np(arr0.dtype),
                              kind="ExternalInput")

    h = {}
    for tag in ("c", "b"):
        rl = pk[tag]
        pc0 = rl["per_core"][0]
        h[f"xt_{tag}"] = din(f"xt_{tag}", pc0["xt"])
        h[f"oh_{tag}"] = din(f"oh_{tag}", pc0["oh"])
        h[f"pa_{tag}"] = din(f"pa_{tag}", pc0["pa"])
        h[f"ohg_{tag}"] = din(f"ohg_{tag}", pc0["ohg"])
        h[f"w2_{tag}"] = din(f"w2_{tag}", rl["w2"])
        h[f"ws4_{tag}"] = din(f"ws4_{tag}", rl["ws4"])
    h["i8"] = din("i8", pk["ident8"])
    h["i16"] = din("i16", pk["ident16"])
    h["recip2"] = din("recip2", pk["recip2"])
    h["zl"] = din("zl", pk["zl"])
    h["zr"] = din("zr", pk["zr"])
    h["z128"] = din("z128", pk["z128"])
    for k, v in pk["mlp"].items():
        h["mlp_" + k] = din("mlp_" + k, v)
    out_h = nc.dram_tensor("out", [1, G], F32, kind="ExternalOutput")

    with tile.TileContext(nc) as tc:
        with tc.tile_pool(name="const", bufs=1) as cp, \
             tc.tile_pool(name="stream", bufs=4) as sp, \
             tc.tile_pool(name="work", bufs=3) as wp, \
             tc.tile_pool(name="svp", bufs=3, space="PSUM") as svp, \
             tc.tile_pool(name="aggp", bufs=2, space="PSUM") as aggp, \
             tc.tile_pool(name="dram", bufs=1, space="DRAM") as dp:

            i8_t = cp.tile([128, 128], FP8, tag="i8")
            nc.sync.dma_start(i8_t[:], h["i8"].ap())
            i16_t = cp.tile([128, 128], F16, tag="i16")
            nc.sync.dma_start(i16_t[:], h["i16"].ap())
            recip_t = cp.tile([128, 128], F16, tag="recip2")
            nc.sync.dma_start(recip_t[:], h["recip2"].ap())
            zl = cp.tile([1, 128], F16, tag="zl")
            nc.sync.dma_start(zl[:], h["zl"].ap())
            zr = cp.tile([1, 512], F16, tag="zr")
            nc.sync.dma_start(zr[:], h["zr"].ap())

            pooled_r = {}
            for rtag in ("c", "b"):
                pooled_r[rtag] = wp.tile([128, 64], F32, name=f"pool{rtag}",
                                         tag=f"pool{rtag}")
                nc.sync.dma_start(pooled_r[rtag][:], h["z128"].ap()[:, 0:64])

            # warm the PE clock (HAM) while the first stream DMAs land
            warm = svp.tile([128, 8, 128], F32, tag="sv")
            for _wi in range(56):
                nc.tensor.matmul(warm[:, _wi % 8, :], i16_t[:], i16_t[:],
                                 start=True, stop=True)

            slab_ctr = [0]

            def relation(tag, col_off):
                rl = pk[tag]
                w2_t = cp.tile([128, 128], F16, tag=f"w2{tag}")
                nc.sync.dma_start(w2_t[:], h[f"w2_{tag}"].ap())
                ws4_t = cp.tile([32, 64], F16, tag=f"ws4{tag}")
                nc.sync.dma_start(ws4_t[:], h[f"ws4_{tag}"].ap())
                xt_v = h[f"xt_{tag}"].ap()
                oh_v = h[f"oh_{tag}"].ap()
                pa_v = h[f"pa_{tag}"].ap()
                ohg_v = h[f"ohg_{tag}"].ap()
                first_pool = [True]
                ngroups = rl["ngroups"]
                pa_w = GRP * 128
                jobs = []
                for g, gr in enumerate(rl["groups"]):
                    if gr["n_b"] == 0:
                        continue
                    for s in range(gr["R"] + gr["F"]):
                        jobs.append((g, gr, s))
                last_g = jobs[-1][0]
                # software pipeline: scatter(slab k) emits after MM1s(k+2)
                # (2-deep: the sigmoid+mult chain is longer than one slab);
                # relu(group) emits with its last scatter; pooling(group)
                # defers one more slab so PE never waits on ACT.
                pend_sc = []
                pend_pool = []     # list of [due_idx, fn]
                cur = [0]

                def flush_sc(depth=1):
                    while len(pend_sc) > depth:
                        pend_sc.pop(0)()

                def flush_pool():
                    while pend_pool and pend_pool[0][0] <= cur[0]:
                        pend_pool.pop(0)[1]()

                def mk_pool(g, gr, agg, ohg_t):
                    n_b = gr["n_b"]
                    h_sb = wp.tile([128, GRP, 64], F16, name=f"h{tag}{g}",
                                   tag="hsb")
                    nc.scalar.activation(h_sb[:, 0:n_b, :],
                                         agg[:, 0:n_b, :], AF.Relu)

                    def pool():
                        gpool = svp.tile([128, 64], F32, name=f"gp{tag}{g}",
                                         tag="sv")
                        for i in range(n_b):
                            nc.tensor.matmul(
                                gpool[:],
                                ohg_t[:, i * 128:(i + 1) * 128],
                                h_sb[:, i, :],
                                start=(i == 0), stop=(i == n_b - 1),
                                skip_group_check=True)
                        nc.vector.tensor_tensor(
                            pooled_r[tag][:], pooled_r[tag][:],
                            gpool[:], op=OP.add)
                    pend_pool.append([cur[0] + 1, pool])

                st = {}
                xt2 = [None, 0]
                for idx, (g, gr, s) in enumerate(jobs):
                    cur[0] = idx
                    n_b, Rs, Fs = gr["n_b"], gr["R"], gr["F"]
                    nsl = Rs + Fs
                    if s == 0:
                        pa_t = sp.tile([32, pa_w], F16, name=f"pa{tag}{g}",
                                       tag="pa")
                        nc.sync.dma_start(pa_t[:],
                                          pa_v[:, g * pa_w:(g + 1) * pa_w])
                        ohg_t = sp.tile([128, GRP * 128], FP8,
                                        name=f"ohg{tag}{g}", tag="ohg")
                        nc.sync.dma_start(
                            ohg_t[:, 0:n_b * 128],
                            ohg_v[:, g * GRP * 128:
                                  g * GRP * 128 + n_b * 128])
                        agg = aggp.tile([128, GRP, 64], F32,
                                        name=f"agg{tag}{g}", tag="agg")
                        nc.tensor.matmul(agg[:, 0:n_b, :], zl[:],
                                         zr[:, 0:n_b * 64], start=True,
                                         stop=False, skip_group_check=True)
                        for i in range(n_b):
                            nc.tensor.matmul(
                                agg[:, i, :],
                                pa_t[0:17, i * 128:i * 128 + 128],
                                ws4_t[0:17, :],
                                start=False, stop=False,
                                skip_group_check=True)
                        nsc = Rs + sum(len(p) for p in gr["flex"])
                        st[g] = {"agg": agg, "ohg_t": ohg_t, "sci": [0],
                                 "nsc": nsc}
                    sg = st[g]
                    agg, ohg_t = sg["agg"], sg["ohg_t"]

                    def sc_flags(sg=sg):
                        sg["sci"][0] += 1
                        return {"start": False,
                                "stop": sg["sci"][0] == sg["nsc"],
                                "skip_group_check": True}

                    # xt DMA batched over slab pairs (within the group)
                    if s % 2 == 0:
                        wcols = min(2, nsl - s) * 1024
                        c0 = gr["xt_off"] * 2 + s * 1024
                        xt2[0] = sp.tile([128, 2048], FP8,
                                         name=f"xt{tag}{g}_{s}", tag="xt")
                        nc.sync.dma_start(xt2[0][:, 0:wcols],
                                          xt_v[:, c0:c0 + wcols])
                    xtsl = xt2[0][:, (s % 2) * 1024:(s % 2) * 1024 + 1024]
                    flex = None
                    if s >= Rs:
                        flex = gr["flex"][s - Rs]
                        ob0 = flex[0][1]
                        obn = len(flex)
                        oh_t = sp.tile([128, GRP * 128], FP8,
                                       name=f"oh{tag}{g}_{s}", tag="ohf")
                        nc.sync.dma_start(
                            oh_t[:, 0:obn * 128],
                            oh_v[:, ob0 * 128:(ob0 + obn) * 128])
                        present = set(i for i, _ in flex)
                    svk = svp.tile([128, 8, 128], F32,
                                   name=f"sv{tag}{g}_{s}", tag="sv")
                    nact = 8 if flex is None else len(flex)
                    for i in range(8):
                        if flex is not None and i not in present:
                            continue
                        nc.tensor.matmul(
                            svk[:, i, :],
                            xtsl[:, i * 128:(i + 1) * 128],
                            w2_t[:],
                            start=True, stop=True)
                    flush_sc(1)  # scatter of slab idx-2 runs behind our MM1s
                    flush_pool()
                    gt = wp.tile([128, 8, 64], F16,
                                 name=f"gt{tag}{g}_{s}", tag="gt")
                    nc.scalar.activation(gt[:, 0:nact, :],
                                         svk[:, 0:nact, 0:64], AF.Sigmoid)
                    msg = wp.tile([128, 8, 64], F16,
                                  name=f"msg{tag}{g}_{s}", tag="msg")
                    nc.vector.tensor_tensor(msg[:, 0:nact, :],
                                            gt[:, 0:nact, :],
                                            svk[:, 0:nact, 64:128],
                                            op=OP.mult)

                    def mk(flex, oh_t, msg, g=g, gr=gr, agg=agg,
                           ohg_t=ohg_t, is_last=(s == nsl - 1),
                           sc_flags=sc_flags):
                        def emit():
                            if flex is None:
                                nc.tensor.matmul(agg[:, 0:gr["n_b"], :],
                                                 i8_t[:],
                                                 msg[:, 0:gr["n_b"], :],
                                                 **sc_flags())
                            else:
                                for j, (i, _) in enumerate(flex):
                                    nc.tensor.matmul(
                                        agg[:, i, :],
                                        oh_t[:, j * 128:(j + 1) * 128],
                                        msg[:, i, :], **sc_flags())
                            if is_last:
                                mk_pool(g, gr, agg, ohg_t)
                        return emit

                    pend_sc.append(mk(flex,
                                      oh_t if flex is not None else None,
                                      msg))
                cur[0] += 1
                flush_sc(0)
                cur[0] += 1
                flush_pool()
                while pend_pool:
                    pend_pool.pop(0)[1]()

            def do_collective(rtag):
                bi = dp.tile([128, 64], F32, name=f"bi{rtag}",
                             tag=f"bi{rtag}")
                bo = dp.tile([128, 64], F32, name=f"bo{rtag}",
                             tag=f"bo{rtag}")
                nc.sync.dma_start(bi[:], pooled_r[rtag][:])
                nc.gpsimd.collective_compute(
                    "AllReduce", OP.add,
                    replica_groups=[list(range(NCORES))],
                    ins=[bi.opt()], outs=[bo.opt()])
                nc.sync.dma_start(pooled_r[rtag][:], bo[:])

            relation("c", 0)
            do_collective("c")   # overlaps relation b's compute
            relation("b", 64)
            do_collective("b")

            # --- head: divide by counts, transpose, MLP ---
            mean16 = wp.tile([128, 128], F16, tag="mean16")
            nc.vector.tensor_tensor(mean16[:, 0:64], pooled_r["c"][:],
                                    recip_t[:, 0:64], op=OP.mult)
            nc.vector.tensor_tensor(mean16[:, 64:128], pooled_r["b"][:],
                                    recip_t[:, 64:128], op=OP.mult)
            tps = aggp.tile([128, 128], F16, tag="agg")
            nc.tensor.transpose(tps[:], mean16[:], i16_t[:])
            mean_sb = wp.tile([128, 128], F16, tag="mean_sb")
            nc.vector.tensor_copy(mean_sb[:], tps[:])

            mw, mb = {}, {}
            for k in ("W1", "W2", "W3", "Wout"):
                mw[k] = cp.tile(list(pk["mlp"][k].shape), F16, name=f"mw{k}",
                                tag=f"mw{k}")
                nc.sync.dma_start(mw[k][:], h["mlp_" + k].ap())
            for k in ("b1", "b2", "b3", "bout"):
                mb[k] = cp.tile(list(pk["mlp"][k].shape), F32, name=f"mb{k}",
                                tag=f"mb{k}")
                nc.sync.dma_start(mb[k][:], h["mlp_" + k].ap())

            hcur = mean_sb
            for li, (wk, bk) in enumerate((("W1", "b1"), ("W2", "b2"),
                                           ("W3", "b3"))):
                ps = aggp.tile([64, G], F32, name=f"mlp{li}", tag="agg")
                nc.tensor.matmul(ps[:], mw[wk][:], hcur[:],
                                 start=True, stop=True)
                hn = wp.tile([64, G], F16, name=f"hn{li}", tag=f"hn{li}")
                nc.scalar.activation(hn[:], ps[:], AF.Relu, bias=mb[bk][:])
                hcur = hn
            ps_o = aggp.tile([1, G], F32, tag="agg")
            nc.tensor.matmul(ps_o[:], mw["Wout"][:], hcur[:],
                             start=True, stop=True)
            osb = wp.tile([1, G], F32, tag="osb")
            nc.scalar.activation(osb[:], ps_o[:], AF.Identity,
                                 bias=mb["bout"][:])
            nc.sync.dma_start(out_h.ap(), osb[:])

    nc.compile()

    in_maps = []
    for m in range(NCORES):
        im = {}
        for tag in ("c", "b"):
            rl = pk[tag]
            pc = rl["per_core"][m]
            im[f"xt_{tag}"] = pc["xt"]
            im[f"oh_{tag}"] = pc["oh"]
            im[f"pa_{tag}"] = pc["pa"]
            im[f"ohg_{tag}"] = pc["ohg"]
            im[f"w2_{tag}"] = rl["w2"]
            im[f"ws4_{tag}"] = rl["ws4"]
        im["i8"] = pk["ident8"]
        im["i16"] = pk["ident16"]
        im["recip2"] = pk["recip2"]
        im["zl"] = pk["zl"]
        im["zr"] = pk["zr"]
        im["z128"] = pk["z128"]
        for k, v in pk["mlp"].items():
            im["mlp_" + k] = v
        in_maps.append(im)

    trace = bool(os.environ.get("KERNEL_TRACE"))
    res = run_bass_kernel_spmd(nc, in_maps, core_ids=list(range(NCORES)),
                               trace=trace)
    global LAST_EXEC_NS
    LAST_EXEC_NS = res.exec_time_ns
    return res.results[0]["out"].reshape(G).astype(np.float32)


# revision 57
# speedup vs baseline: 1.2053x; 1.2053x over previous
"""Trainium2 Bass kernel for hetero-GNN (2x ResGatedGraphConv + segment-mean pooling + MLP).

Sharding: destination-node range per core; each core processes the edges whose
dst falls in its range. Host does index marshalling only; all model arithmetic
runs on device.

Device strategy ("degree rounds"):
  - dst nodes are grouped into 128-slot buckets; 8 buckets form a PSUM group
    whose aggregate [128 slots, 8*64] lives in one PSUM bank.
  - edges of a bucket are packed into R identity rounds (the j-th edge of
    slot p sits at row p of round j) plus <=F flex rounds (leftovers, with a
    host-built fp8 one-hot scatter matrix).
  - a slab = one round of each of the 8 buckets = 8 subtiles of 128 edges.
    Per slab: 8 fused matmuls [xt.T @ W_aug] (row-tiled concurrent pairs),
    one sigmoid (ACT), one gated multiply (DVE), and ONE identity-stationary
    matmul that scatter-adds all 8 subtiles into the group aggregate.
  - skip connection is folded into the same PSUM accumulation (a dummy zero
    matmul opens the accumulation group for the whole bank).
  - relu + pooling via per-bucket one-hot matmul into a [G, 2H] transposed
    pooled PSUM; AllReduce across 8 cores; small MLP head on device.
"""
import os
import sys
import types
import numpy as np
import ml_dtypes

F8NP = ml_dtypes.float8_e4m3fn

NCORES = 8
G = 128
H = 64
F = 16
NC_N = 100000
NB_N = 200000
GRP = 8          # buckets per PSUM group
LAST_EXEC_NS = None


def _install_ntff_shim():
    if 'antenv.axon_hooks' in sys.modules:
        return
    try:
        mod = types.ModuleType('antenv.axon_hooks')
        _h = [None]
        mod.set_axon_ntff_profile_hook = lambda h: _h.__setitem__(0, h)
        mod.get_axon_ntff_profile_hook = lambda: _h[0]
        sys.modules['antenv.axon_hooks'] = mod
        import antenv
        antenv.axon_hooks = mod
        from trn_agent_boot.trn_boot import _ntff_profile_via_ctypes
        mod.set_axon_ntff_profile_hook(
            _ntff_profile_via_ctypes('/opt/axon/libaxon_pjrt.so'))
    except Exception:
        pass


def _waug(ii, rel):
    Wq, Wv, Wk = ii[f"Wq_{rel}"], ii[f"Wv_{rel}"], ii[f"Wk_{rel}"]
    We = ii[f"We_{rel}"][0]
    bq, bv, bk, be = (ii[f"bq_{rel}"], ii[f"bv_{rel}"],
                      ii[f"bk_{rel}"], ii[f"be_{rel}"])
    w = np.zeros((35, 128), np.float32)
    w[0:16, 0:64] = Wq
    w[0:16, 64:128] = Wv
    w[16, 0:64] = 2 * We
    w[16, 64:128] = We
    w[17, 0:64] = bq + bk + 2 * be
    w[17, 64:128] = bv + be
    w[18:34, 0:64] = Wk
    return w


def pack_relation(xs, xd, src, dst, ea, D, ii, rel, batch_dst):
    """Host marshalling for one relation.

    Returns common schedule + per-core device arrays."""
    E = len(src)
    nbuck = (D + 127) // 128
    # per-core degree-sorted slot permutation: rank dst nodes by degree so
    # each 128-slot bucket holds near-equal degrees (minimal round padding).
    core_all = dst // D
    loc_all = dst % D
    deg_node = np.bincount(core_all * D + loc_all,
                           minlength=NCORES * D).reshape(NCORES, D)
    order_nodes = np.argsort(-deg_node, axis=1, kind="stable")  # rank->loc
    rank_of_loc = np.empty((NCORES, D), np.int64)
    np.put_along_axis(rank_of_loc, order_nodes,
                      np.broadcast_to(np.arange(D), (NCORES, D)), axis=1)
    slot_all = rank_of_loc[core_all, loc_all]
    key = core_all * D + slot_all
    order = np.argsort(key, kind="stable")
    src_s, dst_s, ea_s = src[order], dst[order], ea[order]
    key_s = key[order]
    core = core_all[order]
    buck = (key_s % D) // 128
    slot = (key_s % D) % 128
    lin = (core * nbuck + buck) * 128 + slot
    deg = np.bincount(lin, minlength=NCORES * nbuck * 128) \
            .reshape(NCORES, nbuck, 128)
    starts = np.searchsorted(key_s, key_s, side="left")
    rank = np.arange(E) - starts

    # common per-bucket-position R (identity rounds): minimize
    # R + w*max_core(F).  Flex subtiles cost more than identity rounds
    # (extra one-hot DMA + per-subtile scatter matmul), so weight them and
    # prefer the larger R on ties.
    maxd = int(deg.max())
    bestT = np.full(nbuck, np.inf)
    bestR = np.zeros(nbuck, np.int64)
    for R in range(0, maxd + 1):
        lo = np.maximum(deg - R, 0).sum(-1)            # [NCORES, nbuck]
        Fk = (-(-lo // 128)).max(0)                    # [nbuck]
        T = R + 1.8 * Fk
        upd = T <= bestT
        bestT[upd] = T[upd]
        bestR[upd] = R

    # group buckets (sorted by R desc) into chunks of GRP
    border = np.argsort(-bestR, kind="stable")
    ngroups = (nbuck + GRP - 1) // GRP
    groups = []
    bucket_group = np.zeros(nbuck, np.int64)   # bucket -> group
    bucket_pos = np.zeros(nbuck, np.int64)     # bucket -> index in group
    bucket_Rs = np.zeros(nbuck, np.int64)      # bucket -> group R*
    xt_col = 0
    oh_blk = 0
    for g in range(ngroups):
        bks = border[g * GRP:(g + 1) * GRP]
        n_b = len(bks)
        Rs = int(bestR[bks].max()) if n_b else 0
        # leftovers recomputed at group R*; order buckets by flex count so
        # each flex slab's present subtiles form a prefix
        lo2 = np.maximum(deg[:, bks, :] - Rs, 0).sum(-1)   # [NCORES, n_b]
        Fk = (-(-lo2 // 128)).max(0)                       # [n_b]
        perm = np.argsort(-Fk, kind="stable")
        bks = bks[perm]
        Fk = Fk[perm]
        Fs = int(Fk.max()) if n_b else 0
        flex = []
        oh_idx = {}
        for f in range(Fs):
            present = [(i, 0) for i in range(n_b) if Fk[i] > f]
            present = [(i, oh_blk + j) for j, (i, _) in enumerate(present)]
            for i, ob in present:
                oh_idx[(i, f)] = ob
            oh_blk += len(present)
            flex.append(present)
        bucket_group[bks] = g
        bucket_pos[bks] = np.arange(n_b)
        bucket_Rs[bks] = Rs
        groups.append({
            "n_b": n_b, "R": Rs, "F": Fs, "bks": bks, "Fk": Fk,
            "flex": flex, "oh_idx": oh_idx, "xt_off": xt_col,
        })
        xt_col += (Rs + Fs) * 512
    XC = max(xt_col, 512)
    OC = max(oh_blk * 128, 128)

    # per-edge destination column in xt (per core arrays share the schedule)
    # xt layout: [64, nsub*128]; subtile (group g, slab s, pos i) at column
    # block (xt_off/512)*8 + s*8 + i  (xt_off counts 512-col slab units).
    g_of = bucket_group[buck]
    i_of = bucket_pos[buck]
    Rs_of = bucket_Rs[buck]
    xoff_of = np.array([gr["xt_off"] for gr in groups], np.int64)[g_of]
    suboff_of = xoff_of // 512 * 8
    is_id = rank < Rs_of
    col = np.zeros(E, np.int64)
    col[is_id] = ((suboff_of[is_id] + rank[is_id] * 8 + i_of[is_id]) * 128
                  + slot[is_id])
    # flex: position among the bucket's leftover edges (dst-sorted order)
    lx = ~is_id
    lin_lx = lin[lx] // 128      # (core,bucket) linear id of leftover edges
    first = np.searchsorted(lin_lx, lin_lx, side="left")
    fpos = np.arange(lx.sum()) - first
    f_of = fpos // 128
    row = fpos % 128
    col[lx] = ((suboff_of[lx] + (Rs_of[lx] + f_of) * 8 + i_of[lx]) * 128
               + row)
    # oh block index for flex edges
    ohmap = np.full((nbuck, 32), -1, np.int64)
    for gr in groups:
        for (i, f), ob in gr["oh_idx"].items():
            ohmap[gr["bks"][i], f] = ob
    oh_of = np.zeros(E, np.int64)
    oh_of[lx] = ohmap[buck[lx], f_of]
    assert (oh_of[lx] >= 0).all()
    flexrow = np.zeros(E, np.int64)
    flexrow[lx] = row

    xsT = xs.astype(np.float32)
    xdT = xd.astype(np.float32)
    per_core = []
    cb = np.searchsorted(core, np.arange(NCORES + 1))
    for m in range(NCORES):
        s0, s1 = cb[m], cb[m + 1]
        c_src, c_dst = src_s[s0:s1], dst_s[s0:s1]
        c_ea, c_col = ea_s[s0:s1], col[s0:s1]
        c_lx = lx[s0:s1]
        c_oh = oh_of[s0:s1]
        c_fr = flexrow[s0:s1]
        c_slot = slot[s0:s1]
        xt = np.zeros((128, XC * 2), np.float32)
        xt[0:16, c_col] = xsT[c_src].T
        xt[16, c_col] = c_ea
        xt[17, c_col] = 1.0
        xt[18:34, c_col] = xdT[c_dst].T
        # flex one-hots: edge at (oh block, row) -> slot
        oh = np.zeros((128, OC), np.float32)
        oh[c_fr[c_lx], c_oh[c_lx] * 128 + c_slot[c_lx]] = 1.0
        # pa (skip lhsT, [32, nbuck*128]) + ohg (pooling one-hot),
        # columns follow the per-core degree-rank permutation
        PC = ngroups * GRP * 128
        GC = ngroups * GRP * 128
        pa = np.zeros((32, PC), np.float32)
        ohg = np.zeros((128, GC), np.float32)
        rk2loc = order_nodes[m]
        for g, gr in enumerate(groups):
            for i, k in enumerate(gr["bks"]):
                r0 = k * 128
                w = min(128, D - r0)
                if w <= 0:
                    continue
                nodes = m * D + rk2loc[r0:r0 + w]
                cblk = (g * GRP + i) * 128
                pa[0:16, cblk:cblk + w] = xdT[nodes].T
                pa[16, cblk:cblk + w] = 1.0
                bt = batch_dst[nodes]
                ohg[np.arange(w), cblk + bt] = 1.0
        f8 = np.float16 if os.environ.get("KF16") else F8NP
        per_core.append({
            "xt": xt.astype(f8),
            "oh": oh.astype(f8),
            "pa": pa.astype(np.float16),
            "ohg": ohg.astype(f8),
        })

    w2 = np.zeros((128, 128), np.float32)
    w2[0:35] = _waug(ii, rel)
    ws4 = np.zeros((32, 64), np.float32)
    ws4[0:16] = ii[f"Wskip_{rel}"]
    ws4[16] = ii[f"bconv_{rel}"]
    nslabs = sum(gr["R"] + gr["F"] for gr in groups)
    return {
        "groups": groups, "XC": XC, "OC": OC, "ngroups": ngroups,
        "per_core": per_core, "w2": w2.astype(np.float16),
        "ws4": ws4.astype(np.float16), "nslabs": nslabs, "D": D,
        "nbuck": nbuck,
    }


def pack_all(ii):
    Dc, Db = NC_N // NCORES, NB_N // NCORES
    rel_c = pack_relation(ii["x_x"], ii["x_c"], ii["src_ac"].astype(np.int64),
                          ii["dst_ac"].astype(np.int64),
                          np.asarray(ii["ea_ac"])[:, 0], Dc, ii, "ac",
                          ii["batch_c"].astype(np.int64))
    rel_b = pack_relation(ii["x_c"], ii["x_b"], ii["src_cb"].astype(np.int64),
                          ii["dst_cb"].astype(np.int64),
                          np.asarray(ii["ea_cb"])[:, 0], Db, ii, "cb",
                          ii["batch_b"].astype(np.int64))

    cnt_c = np.bincount(ii["batch_c"].astype(np.int64), minlength=G)
    cnt_b = np.bincount(ii["batch_b"].astype(np.int64), minlength=G)
    recip2 = np.zeros((G, 128), np.float32)
    recip2[:, 0:64] = (1.0 / np.maximum(cnt_c, 1))[:, None]
    recip2[:, 64:128] = (1.0 / np.maximum(cnt_b, 1))[:, None]

    mlp = {
        "W1": ii["W1"].astype(np.float16), "W2": ii["W2"].astype(np.float16),
        "W3": ii["W3"].astype(np.float16),
        "Wout": ii["Wout"].astype(np.float16),
        "b1": np.asarray(ii["b1"], np.float32).reshape(64, 1),
        "b2": np.asarray(ii["b2"], np.float32).reshape(64, 1),
        "b3": np.asarray(ii["b3"], np.float32).reshape(64, 1),
        "bout": np.asarray(ii["bout"], np.float32).reshape(1, 1),
    }
    f8 = np.float16 if os.environ.get("KF16") else F8NP
    ident8 = np.eye(128, dtype=f8)
    ident16 = np.eye(128, dtype=np.float16)
    zl = np.zeros((1, 128), np.float16)
    zr = np.zeros((1, 512), np.float16)
    z128 = np.zeros((128, 128), np.float32)
    return {"c": rel_c, "b": rel_b, "recip2": recip2.astype(np.float16),
            "mlp": mlp, "ident8": ident8, "ident16": ident16,
            "zl": zl, "zr": zr, "z128": z128}


def emulate(ii):
    """Numpy emulation of the device program (for packing validation)."""
    pk = pack_all(ii)
    pooled = np.zeros((G, 128), np.float64)
    for tag in ("c", "b"):
        rl = pk[tag]
        w2 = rl["w2"].astype(np.float32)
        ws4 = rl["ws4"].astype(np.float32)
        for m in range(NCORES):
            pc = rl["per_core"][m]
            xt = pc["xt"].astype(np.float32)
            oh = pc["oh"].astype(np.float32)
            pa = pc["pa"].astype(np.float32)
            ohg = pc["ohg"].astype(np.float32)
            for g, gr in enumerate(rl["groups"]):
                n_b, Rs, Fs = gr["n_b"], gr["R"], gr["F"]
                agg = np.zeros((128, n_b, 64), np.float32)
                for i in range(n_b):
                    cblk = (g * GRP + i) * 128
                    lhs = pa[0:17, cblk:cblk + 128]
                    agg[:, i, :] += lhs.T @ ws4[0:17]
                for s in range(Rs + Fs):
                    c0 = gr["xt_off"] * 2 + s * 1024
                    blkx = xt[:, c0:c0 + 1024]
                    sv = np.zeros((128, 8, 128), np.float32)
                    for i in range(8):
                        sv[:, i, :] = (blkx[:, i * 128:(i + 1) * 128].T @ w2)
                    gt = (1.0 / (1.0 + np.exp(-sv[:, :, 0:64]))) \
                        .astype(np.float16).astype(np.float32)
                    msg = (gt * sv[:, :, 64:128]).astype(np.float16) \
                        .astype(np.float32)
                    if s < Rs:
                        agg += msg[:, :n_b, :]
                    else:
                        for (i, ob) in gr["flex"][s - Rs]:
                            ohb = oh[:, ob * 128:(ob + 1) * 128]
                            agg[:, i, :] += ohb.T @ msg[:, i, :]
                h = np.maximum(agg, 0.0).astype(np.float16).astype(np.float32)
                off = 0 if tag == "c" else 64
                for i in range(n_b):
                    ohgb = ohg[:, (g * GRP + i) * 128:(g * GRP + i + 1) * 128]
                    pooled[:, off:off + 64] += ohgb.T @ h[:, i, :]
    mean = pooled * pk["recip2"].astype(np.float64)
    hcur = mean.T.astype(np.float32)          # [2H, G]
    mlp = pk["mlp"]
    for wk, bk in (("W1", "b1"), ("W2", "b2"), ("W3", "b3")):
        hcur = np.maximum(mlp[wk].astype(np.float32).T @ hcur + mlp[bk], 0.0)
    out = mlp["Wout"].astype(np.float32).T @ hcur + mlp["bout"]
    return out.reshape(G)


def kernel(**inputs):
    _install_ntff_shim()
    import concourse.bass as bass  # noqa: F401
    import concourse.bacc as bacc
    import concourse.mybir as mybir
    import concourse.tile as tile
    from concourse.bass_utils import run_bass_kernel_spmd

    F32 = mybir.dt.float32
    F16 = mybir.dt.float16
    FP8 = F16 if os.environ.get("KF16") else mybir.dt.float8e4
    AF = mybir.ActivationFunctionType
    OP = mybir.AluOpType

    ii = {k: np.asarray(v) for k, v in inputs.items()}
    pk = pack_all(ii)

    nc = bacc.Bacc("TRN2", target_bir_lowering=False, debug=False,
                   num_devices=NCORES)

    def din(name, arr0):
        return nc.dram_tensor(name, list(arr0.shape),
                              mybir.dt.from_np(arr0.dtype),
                              kind="ExternalInput")

    h = {}
    for tag in ("c", "b"):
        rl = pk[tag]
        pc0 = rl["per_core"][0]
        h[f"xt_{tag}"] = din(f"xt_{tag}", pc0["xt"])
        h[f"oh_{tag}"] = din(f"oh_{tag}", pc0["oh"])
        h[f"pa_{tag}"] = din(f"pa_{tag}", pc0["pa"])
        h[f"ohg_{tag}"] = din(f"ohg_{tag}", pc0["ohg"])
        h[f"w2_{tag}"] = din(f"w2_{tag}", rl["w2"])
        h[f"ws4_{tag}"] = din(f"ws4_{tag}", rl["ws4"])
    h["i8"] = din("i8", pk["ident8"])
    h["i16"] = din("i16", pk["ident16"])
    h["recip2"] = din("recip2", pk["recip2"])
    h["zl"] = din("zl", pk["zl"])
    h["zr"] = din("zr", pk["zr"])
    h["z128"] = din("z128", pk["z128"])
    for k, v in pk["mlp"].items():
        h["mlp_" + k] = din("mlp_" + k, v)
    out_h = nc.dram_tensor("out", [1, G], F32, kind="ExternalOutput")

    with tile.TileContext(nc) as tc:
        with tc.tile_pool(name="const", bufs=1) as cp, \
             tc.tile_pool(name="stream", bufs=4) as sp, \
             tc.tile_pool(name="work", bufs=3) as wp, \
             tc.tile_pool(name="svp", bufs=3, space="PSUM") as svp, \
             tc.tile_pool(name="aggp", bufs=2, space="PSUM") as aggp, \
             tc.tile_pool(name="dram", bufs=1, space="DRAM") as dp:

            i8_t = cp.tile([128, 128], FP8, tag="i8")
            nc.sync.dma_start(i8_t[:], h["i8"].ap())
            i16_t = cp.tile([128, 128], F16, tag="i16")
            nc.sync.dma_start(i16_t[:], h["i16"].ap())
            recip_t = cp.tile([128, 128], F16, tag="recip2")
            nc.sync.dma_start(recip_t[:], h["recip2"].ap())
            zl = cp.tile([1, 128], F16, tag="zl")
            nc.sync.dma_start(zl[:], h["zl"].ap())
            zr = cp.tile([1, 512], F16, tag="zr")
            nc.sync.dma_start(zr[:], h["zr"].ap())

            pooled_r = {}
            for rtag in ("c", "b"):
                pooled_r[rtag] = wp.tile([128, 64], F32, name=f"pool{rtag}",
                                         tag=f"pool{rtag}")
                nc.sync.dma_start(pooled_r[rtag][:], h["z128"].ap()[:, 0:64])

            # warm the PE clock (HAM) while the first stream DMAs land
            warm = svp.tile([128, 8, 128], F32, tag="sv")
            for _wi in range(56):
                nc.tensor.matmul(warm[:, _wi % 8, :], i16_t[:], i16_t[:],
                                 start=True, stop=True)

            slab_ctr = [0]

            def relation(tag, col_off):
                rl = pk[tag]
                w2_t = cp.tile([128, 128], F16, tag=f"w2{tag}")
                nc.sync.dma_start(w2_t[:], h[f"w2_{tag}"].ap())
                ws4_t = cp.tile([32, 64], F16, tag=f"ws4{tag}")
                nc.sync.dma_start(ws4_t[:], h[f"ws4_{tag}"].ap())
                xt_v = h[f"xt_{tag}"].ap()
                oh_v = h[f"oh_{tag}"].ap()
                pa_v = h[f"pa_{tag}"].ap()
                ohg_v = h[f"ohg_{tag}"].ap()
                first_pool = [True]
                ngroups = rl["ngroups"]
                pa_w = GRP * 128
                jobs = []
                for g, gr in enumerate(rl["groups"]):
                    if gr["n_b"] == 0:
                        continue
                    for s in range(gr["R"] + gr["F"]):
                        jobs.append((g, gr, s))
                last_g = jobs[-1][0]
                # software pipeline: scatter(slab k) emits after MM1s(k+2)
                # (2-deep: the sigmoid+mult chain is longer than one slab);
                # relu(group) emits with its last scatter; pooling(group)
                # defers one more slab so PE never waits on ACT.
                pend_sc = []
                pend_pool = []     # list of [due_idx, fn]
                cur = [0]

                def flush_sc(depth=1):
                    while len(pend_sc) > depth:
                        pend_sc.pop(0)()

                def flush_pool():
                    while pend_pool and pend_pool[0][0] <= cur[0]:
                        pend_pool.pop(0)[1]()

                def mk_pool(g, gr, agg, ohg_t):
                    n_b = gr["n_b"]
                    h_sb = wp.tile([128, GRP, 64], F16, name=f"h{tag}{g}",
                                   tag="hsb")
                    nc.scalar.activation(h_sb[:, 0:n_b, :],
                                         agg[:, 0:n_b, :], AF.Relu)

                    def pool():
                        gpool = svp.tile([128, 64], F32, name=f"gp{tag}{g}",
                                         tag="sv")
                        for i in range(n_b):
                            nc.tensor.matmul(
                                gpool[:],
                                ohg_t[:, i * 128:(i + 1) * 128],
                                h_sb[:, i, :],
                                start=(i == 0), stop=(i == n_b - 1),
                                skip_group_check=True)
                        nc.vector.tensor_tensor(
                            pooled_r[tag][:], pooled_r[tag][:],
                            gpool[:], op=OP.add)
                    pend_pool.append([cur[0] + 1, pool])

                st = {}
                xt2 = [None, 0]
                for idx, (g, gr, s) in enumerate(jobs):
                    cur[0] = idx
                    n_b, Rs, Fs = gr["n_b"], gr["R"], gr["F"]
                    nsl = Rs + Fs
                    if s == 0:
                        pa_t = sp.tile([32, pa_w], F16, name=f"pa{tag}{g}",
                                       tag="pa")
                        nc.sync.dma_start(pa_t[:],
                                          pa_v[:, g * pa_w:(g + 1) * pa_w])
                        ohg_t = sp.tile([128, GRP * 128], FP8,
                                        name=f"ohg{tag}{g}", tag="ohg")
                        nc.sync.dma_start(
                            ohg_t[:, 0:n_b * 128],
                            ohg_v[:, g * GRP * 128:
                                  g * GRP * 128 + n_b * 128])
                        agg = aggp.tile([128, GRP, 64], F32,
                                        name=f"agg{tag}{g}", tag="agg")
                        nc.tensor.matmul(agg[:, 0:n_b, :], zl[:],
                                         zr[:, 0:n_b * 64], start=True,
                                         stop=False, skip_group_check=True)
                        for i in range(n_b):
                            nc.tensor.matmul(
                                agg[:, i, :],
                                pa_t[0:17, i * 128:i * 128 + 128],
                                ws4_t[0:17, :],
                                start=False, stop=False,
                                skip_group_check=True)
                        nsc = Rs + sum(len(p) for p in gr["flex"])
                        st[g] = {"agg": agg, "ohg_t": ohg_t, "sci": [0],
                                 "nsc": nsc}
                    sg = st[g]
                    agg, ohg_t = sg["agg"], sg["ohg_t"]

                    def sc_flags(sg=sg):
                        sg["sci"][0] += 1
                        return {"start": False,
                                "stop": sg["sci"][0] == sg["nsc"],
                                "skip_group_check": True}

                    # xt DMA batched over slab pairs (within the group)
                    if s % 2 == 0:
                        wcols = min(2, nsl - s) * 1024
                        c0 = gr["xt_off"] * 2 + s * 1024
                        xt2[0] = sp.tile([128, 2048], FP8,
                                         name=f"xt{tag}{g}_{s}", tag="xt")
                        nc.sync.dma_start(xt2[0][:, 0:wcols],
                                          xt_v[:, c0:c0 + wcols])
                    xtsl = xt2[0][:, (s % 2) * 1024:(s % 2) * 1024 + 1024]
                    flex = None
                    if s >= Rs:
                        flex = gr["flex"][s - Rs]
                        ob0 = flex[0][1]
                        obn = len(flex)
                        oh_t = sp.tile([128, GRP * 128], FP8,
                                       name=f"oh{tag}{g}_{s}", tag="ohf")
                        nc.sync.dma_start(
                            oh_t[:, 0:obn * 128],
                            oh_v[:, ob0 * 128:(ob0 + obn) * 128])
                        present = set(i for i, _ in flex)
                    svk = svp.tile([128, 8, 128], F32,
                                   name=f"sv{tag}{g}_{s}", tag="sv")
                    nact = 8 if flex is None else len(flex)
                    for i in range(8):
                        if flex is not None and i not in present:
                            continue
                        nc.tensor.matmul(
                            svk[:, i, :],
                            xtsl[:, i * 128:(i + 1) * 128],
                            w2_t[:],
                            start=True, stop=True)
                    flush_sc(1)  # scatter of slab idx-2 runs behind our MM1s
                    flush_pool()
                    gt = wp.tile([128, 8, 64], F16,
                                 name=f"gt{tag}{g}_{s}", tag="gt")
                    nc.scalar.activation(gt[:, 0:nact, :],
                                         svk[:, 0:nact, 0:64], AF.Sigmoid)
                    msg = wp.tile([128, 8, 64], F16,
                                  name=f"msg{tag}{g}_{s}", tag="msg")
                    nc.vector.tensor_tensor(msg[:, 0:nact, :],
                                            gt[:, 0:nact, :],
                                            svk[:, 0:nact, 64:128],
                                            op=OP.mult)

                    def mk(flex, oh_t, msg, g=g, gr=gr, agg=agg,
                           ohg_t=ohg_t, is_last=(s == nsl - 1),
                           sc_flags=sc_flags):
                        def emit():
                            if flex is None:
                                nc.tensor.matmul(agg[:, 0:gr["n_b"], :],
                                                 i8_t[:],
                                                 msg[:, 0:gr["n_b"], :],
                                                 **sc_flags())
                            else:
                                for j, (i, _) in enumerate(flex):
                                    nc.tensor.matmul(
                                        agg[:, i, :],
                                        oh_t[:, j * 128:(j + 1) * 128],
                                        msg[:, i, :], **sc_flags())
                            if is_last:
                                mk_pool(g, gr, agg, ohg_t)
                        return emit

                    pend_sc.append(mk(flex,
                                      oh_t if flex is not None else None,
                                      msg))
                cur[0] += 1
                flush_sc(0)
                cur[0] += 1
                flush_pool()
                while pend_pool:
                    pend_pool.pop(0)[1]()

            bounce = {}

            def start_collective(rtag):
                bi = dp.tile([128, 64], F32, name=f"bi{rtag}",
                             tag=f"bi{rtag}")
                bo = dp.tile([128, 64], F32, name=f"bo{rtag}",
                             tag=f"bo{rtag}")
                nc.sync.dma_start(bi[:], pooled_r[rtag][:])
                nc.gpsimd.collective_compute(
                    "AllReduce", OP.add,
                    replica_groups=[list(range(NCORES))],
                    ins=[bi.opt()], outs=[bo.opt()])
                bounce[rtag] = bo

            relation("c", 0)
            start_collective("c")   # overlaps relation b's compute
            relation("b", 64)
            start_collective("b")
            # back-DMAs emitted only now: a dma waiting on the collective
            # would block the Sync engine's FIFO (and every stream DMA
            # queued behind it)
            nc.sync.dma_start(pooled_r["c"][:], bounce["c"][:])
            nc.sync.dma_start(pooled_r["b"][:], bounce["b"][:])

            # --- head: divide by counts, transpose, MLP ---
            mean16 = wp.tile([128, 128], F16, tag="mean16")
            nc.vector.tensor_tensor(mean16[:, 0:64], pooled_r["c"][:],
                                    recip_t[:, 0:64], op=OP.mult)
            nc.vector.tensor_tensor(mean16[:, 64:128], pooled_r["b"][:],
                                    recip_t[:, 64:128], op=OP.mult)
            tps = aggp.tile([128, 128], F16, tag="agg")
            nc.tensor.transpose(tps[:], mean16[:], i16_t[:])
            mean_sb = wp.tile([128, 128], F16, tag="mean_sb")
            nc.vector.tensor_copy(mean_sb[:], tps[:])

            mw, mb = {}, {}
            for k in ("W1", "W2", "W3", "Wout"):
                mw[k] = cp.tile(list(pk["mlp"][k].shape), F16, name=f"mw{k}",
                                tag=f"mw{k}")
                nc.sync.dma_start(mw[k][:], h["mlp_" + k].ap())
            for k in ("b1", "b2", "b3", "bout"):
                mb[k] = cp.tile(list(pk["mlp"][k].shape), F32, name=f"mb{k}",
                                tag=f"mb{k}")
                nc.sync.dma_start(mb[k][:], h["mlp_" + k].ap())

            hcur = mean_sb
            for li, (wk, bk) in enumerate((("W1", "b1"), ("W2", "b2"),
                                           ("W3", "b3"))):
                ps = aggp.tile([64, G], F32, name=f"mlp{li}", tag="agg")
                nc.tensor.matmul(ps[:], mw[wk][:], hcur[:],
                                 start=True, stop=True)
                hn = wp.tile([64, G], F16, name=f"hn{li}", tag=f"hn{li}")
                nc.scalar.activation(hn[:], ps[:], AF.Relu, bias=mb[bk][:])
                hcur = hn
            ps_o = aggp.tile([1, G], F32, tag="agg")
            nc.tensor.matmul(ps_o[:], mw["Wout"][:], hcur[:],
                             start=True, stop=True)
            osb = wp.tile([1, G], F32, tag="osb")
            nc.scalar.activation(osb[:], ps_o[:], AF.Identity,
                                 bias=mb["bout"][:])
            nc.sync.dma_start(out_h.ap(), osb[:])

    nc.compile()

    in_maps = []
    for m in range(NCORES):
        im = {}
        for tag in ("c", "b"):
            rl = pk[tag]
            pc = rl["per_core"][m]
            im[f"xt_{tag}"] = pc["xt"]
            im[f"oh_{tag}"] = pc["oh"]
            im[f"pa_{tag}"] = pc["pa"]
            im[f"ohg_{tag}"] = pc["ohg"]
            im[f"w2_{tag}"] = rl["w2"]
            im[f"ws4_{tag}"] = rl["ws4"]
        im["i8"] = pk["ident8"]
        im["i16"] = pk["ident16"]
        im["recip2"] = pk["recip2"]
        im["zl"] = pk["zl"]
        im["zr"] = pk["zr"]
        im["z128"] = pk["z128"]
        for k, v in pk["mlp"].items():
            im["mlp_" + k] = v
        in_maps.append(im)

    trace = bool(os.environ.get("KERNEL_TRACE"))
    res = run_bass_kernel_spmd(nc, in_maps, core_ids=list(range(NCORES)),
                               trace=trace)
    global LAST_EXEC_NS
    LAST_EXEC_NS = res.exec_time_ns
    return res.results[0]["out"].reshape(G).astype(np.float32)
